# revision 19
# baseline (speedup 1.0000x reference)
"""Trainium2 Bass kernel for a 2-layer mean-aggregation GNN + MLP head.

Strategy (8 NeuronCores, SPMD single program):
  - Nodes are sharded by dst row: core c owns rows [c*6250, (c+1)*6250).
    Self-edges (i -> i) are appended so (x + agg) and (1 + cnt) come out
    of the aggregation directly; 1/(1+deg) scales are graph metadata and
    are precomputed host-side with the rest of the edge partitioning.
  - Per core, edges are grouped by (dst block of 128 rows, src half) and
    padded to multiples of 128 so every core runs an identical
    instruction stream (chunk counts max'd across cores; pad edges
    gather row 0 and have dstloc=-1 so their one-hot row is all zero).
  - Gather of x[src] uses gpsimd.dma_gather (256B bf16 rows), 4 SWDGE
    queues round-robin. This Q7 descriptor generation is the kernel's
    floor (~4.3ns/edge).
  - Scatter/segment-sum runs on the TensorEngine: per 128-edge chunk, a
    one-hot Pd[e, d] is the stationary operand and gathered messages
    stream, accumulating [128 dst, 64] tiles in PSUM (one block per
    bank; interleaved accumulation groups within a bank corrupt PSUM).
  - One-hots for the whole layer are built on the VectorEngine into one
    persistent tile up front; DVE work concurrent with SWDGE desc-gen
    runs ~20x slow (shared SBUF port), so everything else avoids DVE.
  - Per-block epilogue is ScalarE+PE only: u = agg*s (ACT copy w/ scale),
    PE transpose, the 64x64 layer matmul (weights augmented with a
    constant-one row so classifier biases ride inside the matmuls),
    Prelu (leaky relu) with BN folded into its scale/bias, transpose
    back. Between layers an AllGather rebuilds the full bf16 table.
"""

import sys

sys.path.insert(0, "/opt/trn_rl_repo")

import numpy as np
import ml_dtypes

N = 50000
E = 800000
D = 64
C = 8
NPC = N // C          # 6250 dst rows per core
BLK = 128
NBLK = (NPC + BLK - 1) // BLK   # 49
NPAD = NBLK * BLK               # 6272
HALF = N // 2                   # 25000 (int16 gather index limit)
SGB = 7                         # dst blocks per gather-call group
NSG = (NBLK + SGB - 1) // SGB   # 7
G_ONEHOT = 16                   # chunks per one-hot build batch
NEG_SLOPE = 0.2
BN_EPS = 1e-5

_compiled = {}
DEBUG = 0


def _preprocess(edge_index):
    """Partition/sort/pad edges (incl. self-edges); build per-core gather
    indices, block-local dst values, 1/deg scales, and the shared chunk
    structure."""
    src0 = edge_index[0].astype(np.int64)
    dst0 = edge_index[1].astype(np.int64)
    self_ix = np.arange(N, dtype=np.int64)
    src = np.concatenate([src0, self_ix])
    dst = np.concatenate([dst0, self_ix])
    core = dst // NPC
    dloc = dst - core * NPC
    block = dloc // BLK
    half = (src >= HALF).astype(np.int64)

    # 1/(1 + cnt) per node (self-edge included in the histogram)
    deg = np.bincount(dst, minlength=N).astype(np.float32)
    s = 1.0 / deg
    s_t = np.ones((C, 128, NBLK), np.float32)
    sv = s.reshape(C, NPC)
    for c in range(C):
        full = (NPC // 128) * 128
        s_t[c, :, :NPC // 128] = sv[c, :full].reshape(-1, 128).T
        s_t[c, :NPC - full, NPC // 128] = sv[c, full:]

    key = (block * 2 + half) * C + core
    counts = np.bincount(key, minlength=NBLK * 2 * C).reshape(NBLK, 2, C)
    K = np.ceil(counts / BLK).astype(np.int64).max(axis=2)  # [NBLK, 2]
    K[:, 0] = np.maximum(K[:, 0], 1)

    order = []
    for sg in range(NSG):
        blocks = range(sg * SGB, min((sg + 1) * SGB, NBLK))
        for h in (0, 1):
            for b in blocks:
                order.append((b, h))
    group_off = {}
    nch = 0
    for (b, h) in order:
        group_off[(b, h)] = nch
        nch += int(K[b, h])

    idx_all = np.zeros((C, nch * BLK), np.int16)
    dl_all = np.full((C, nch * BLK), -1.0, np.float32)
    sort_key = ((core * NBLK + block) * 2 + half)
    perm = np.argsort(sort_key, kind="stable")
    s_src, s_dl, s_key = src[perm], dloc[perm], sort_key[perm]
    starts = np.searchsorted(s_key, np.arange(C * NBLK * 2))
    ends = np.searchsorted(s_key, np.arange(C * NBLK * 2), side="right")
    for c in range(C):
        for (b, h) in order:
            k = (c * NBLK + b) * 2 + h
            a, e = int(starts[k]), int(ends[k])
            n = e - a
            o = group_off[(b, h)] * BLK
            idx_all[c, o:o + n] = (s_src[a:e] - h * HALF).astype(np.int16)
            dl_all[c, o:o + n] = (s_dl[a:e] - b * BLK).astype(np.float32)

    idx_t = np.ascontiguousarray(
        np.tile(idx_all.reshape(C, -1, 16).transpose(0, 2, 1), (1, 8, 1)))
    dl_t = np.ascontiguousarray(
        dl_all.reshape(C, nch, BLK).transpose(0, 2, 1)).astype(ml_dtypes.bfloat16)

    calls = []
    for sg in range(NSG):
        blocks = list(range(sg * SGB, min((sg + 1) * SGB, NBLK)))
        for h in (0, 1):
            n = int(sum(K[b, h] for b in blocks))
            if n == 0:
                continue
            calls.append((group_off[(blocks[0], h)], n, h))

    chunk_meta = []
    last_of = {}
    first_of = {}
    for (b, h) in order:
        for j in range(int(K[b, h])):
            ci = group_off[(b, h)] + j
            if b not in first_of:
                first_of[b] = ci
            last_of[b] = ci
            chunk_meta.append(b)
    return dict(nch=nch, K=K, calls=calls, chunk_meta=chunk_meta,
                first_of=first_of, last_of=last_of,
                idx_t=idx_t, dl_t=dl_t, s_t=s_t)


def _build(meta):
    import concourse.bacc as bacc
    import concourse.mybir as mybir
    from concourse import tile
    from concourse.bass import AP

    dt = mybir.dt
    F32, BF16, I16 = dt.float32, dt.bfloat16, dt.int16
    F8 = dt.float8e4
    Act = mybir.ActivationFunctionType
    nch = meta["nch"]
    calls = meta["calls"]
    chunk_meta = meta["chunk_meta"]
    first_of, last_of = meta["first_of"], meta["last_of"]

    nc = bacc.Bacc("TRN2", target_bir_lowering=False, debug=False,
                   num_devices=C, num_swdge_queues=4)

    xpad_d = nc.dram_tensor("xpad", [N, 128], BF16, kind="ExternalInput")
    idx_d = nc.dram_tensor("idx", [128, nch * 8], I16, kind="ExternalInput")
    dl_d = nc.dram_tensor("dl", [128, nch], BF16, kind="ExternalInput")
    iota_d = nc.dram_tensor("iota", [128, G_ONEHOT * 128], BF16, kind="ExternalInput")
    ident_d = nc.dram_tensor("ident", [128, 128], BF16, kind="ExternalInput")
    wb_d = nc.dram_tensor("wb", [65, 168], BF16, kind="ExternalInput")
    vec_d = nc.dram_tensor("vecs", [65, 8], F32, kind="ExternalInput")
    s_d = nc.dram_tensor("s", [128, NBLK], F32, kind="ExternalInput")
    out_d = nc.dram_tensor("out", [128, 2 * NBLK], F32, kind="ExternalOutput")
    if DEBUG:
        dbga_d = nc.dram_tensor("dbg_agg", [128, NBLK * 64], F32, kind="ExternalOutput")

    with tile.TileContext(nc) as tc:
        with tc.tile_pool(name="dram", bufs=1, space="DRAM") as dram, \
             tc.tile_pool(name="const", bufs=1) as constp, \
             tc.tile_pool(name="persist", bufs=1) as pers, \
             tc.tile_pool(name="msg", bufs=2) as msgp, \
             tc.tile_pool(name="onehot", bufs=1) as ohp, \
             tc.tile_pool(name="ep", bufs=4) as epp, \
             tc.tile_pool(name="ps_agg", bufs=5, space="PSUM") as ps_agg, \
             tc.tile_pool(name="ps_tr", bufs=1, space="PSUM") as ps_tr, \
             tc.tile_pool(name="ps_f", bufs=2, space="PSUM") as ps_f:

            z1b = dram.tile([NPC, 128], BF16)
            z1f = dram.tile([N, 128], BF16, addr_space="Shared")

            idx_t = constp.tile([128, nch * 8], I16)
            dl_t = constp.tile([128, nch], BF16)
            iota_t = constp.tile([128, G_ONEHOT * 128], BF16)
            ident_t = constp.tile([128, 128], BF16)
            wb_t = constp.tile([65, 168], BF16)
            vec_t = constp.tile([65, 8], F32)
            s_t = constp.tile([128, NBLK], F32)
            nc.sync.dma_start(idx_t[:], idx_d[:])
            nc.sync.dma_start(dl_t[:], dl_d[:])
            nc.sync.dma_start(iota_t[:], iota_d[:])
            nc.sync.dma_start(ident_t[:], ident_d[:])
            nc.sync.dma_start(wb_t[:], wb_d[:])
            nc.sync.dma_start(vec_t[:], vec_d[:])
            nc.sync.dma_start(s_t[:], s_d[:])

            zstage = pers.tile([128, NBLK * D], BF16)
            out_sb = pers.tile([128, 2 * NBLK], F32)
            if DEBUG:
                dbga_t = pers.tile([128, NBLK * 64], F32)

            for layer in range(2):
                gsrc = xpad_d if layer == 0 else z1f
                Wl = wb_t[0:64, layer * 65:layer * 65 + 65]
                bl = vec_t[:, layer * 3 + 0:layer * 3 + 1]
                al = vec_t[:, layer * 3 + 1:layer * 3 + 2]
                cl = vec_t[:, layer * 3 + 2:layer * 3 + 3]
                Wc1a = wb_t[0:65, 130:163]
                Wc2a = wb_t[0:33, 163:165]

                # ---- one-hots for the whole layer, built up front ----
                P_t = ohp.tile([128, nch * 128], F8, tag="P", name=f"P{layer}")
                for g0 in range(0, nch, G_ONEHOT):
                    g = min(G_ONEHOT, nch - g0)
                    dsl = dl_t[:, g0:g0 + g]
                    dl_b = AP(dsl.tensor, dsl.offset,
                              [dsl.ap[0], [dsl.ap[1][0], g], [0, 128]])
                    nc.vector.tensor_tensor(
                        P_t[:, g0 * 128:(g0 + g) * 128],
                        iota_t[:, 0:g * 128], dl_b, mybir.AluOpType.is_equal)

                # ---- gather calls (SWDGE, 4 queues round-robin) ----
                msg_tiles = {}
                for gi, (c0, n, h) in enumerate(calls):
                    m = msgp.tile([128, n, 128], BF16, tag="msg",
                                  name=f"m{layer}_{gi}")
                    src_ap = gsrc[h * HALF:h * HALF + HALF, :]
                    nc.gpsimd.dma_gather(
                        m[:], src_ap, idx_t[:, c0 * 8:(c0 + n) * 8],
                        n * 128, n * 128, 128,
                        single_packet=False, queue_num=gi % 4)
                    for k in range(n):
                        msg_tiles[c0 + k] = (m, k)

                # ---- scatter matmuls + per-block epilogues ----
                blk_tiles = {}
                for ci in range(nch):
                    b = chunk_meta[ci]
                    if b not in blk_tiles:
                        blk_tiles[b] = ps_agg.tile(
                            [128, 64], F32, tag="agg", name=f"agg{layer}_{b}")
                    m, k = msg_tiles[ci]
                    nc.tensor.matmul(
                        blk_tiles[b][:],
                        P_t[:, ci * 128:(ci + 1) * 128], m[:, k, 0:64],
                        start=(ci == first_of[b]), stop=(ci == last_of[b]))

                    if ci != last_of[b]:
                        continue

                    agg = blk_tiles[b][:]
                    if DEBUG and layer == 0:
                        nc.vector.tensor_copy(
                            dbga_t[:, b * 64:(b + 1) * 64], agg)
                    # u = agg * (1/deg)  (per-partition scale on ACT)
                    u = epp.tile([128, D], BF16, tag="u")
                    nc.scalar.activation(u[:], agg, Act.Copy,
                                         scale=s_t[:, b:b + 1])
                    tr_ps = ps_tr.tile([128, 256], BF16, tag="tr",
                                       name=f"tr{layer}_{b}")
                    nc.tensor.transpose(tr_ps[0:64, 0:128], u[:], ident_t[:])
                    uT = epp.tile([64, 128], BF16, tag="uT")
                    nc.scalar.activation(uT[:], tr_ps[0:64, 0:128], Act.Copy)
                    f_ps = ps_f.tile([128, 512], F32, tag="f",
                                     name=f"f{layer}_{b}")
                    z_ps = f_ps[0:65, 0:128]
                    nc.tensor.matmul(z_ps, Wl, uT[:], start=True, stop=True)
                    t1 = epp.tile([65, 128], F32, tag="t1")
                    nc.scalar.activation(t1[:], z_ps, Act.Prelu,
                                         bias=bl, alpha=NEG_SLOPE)
                    zT = epp.tile([65, 128], BF16, tag="zT")
                    nc.scalar.activation(zT[:], t1[:], Act.Prelu,
                                         scale=al, bias=cl, alpha=NEG_SLOPE)
                    if layer == 0:
                        trb = tr_ps[0:128, 128:192]
                        nc.tensor.transpose(trb, zT[0:64, :],
                                            ident_t[0:64, 0:64])
                        nc.scalar.activation(
                            zstage[:, b * D:(b + 1) * D], trb, Act.Copy)
                    else:
                        q_ps = f_ps[0:33, 128:256]
                        nc.tensor.matmul(q_ps, Wc1a, zT[:],
                                         start=True, stop=True)
                        qs = epp.tile([33, 128], BF16, tag="qs")
                        nc.scalar.activation(qs[:], q_ps, Act.Relu)
                        y_ps = f_ps[0:128, 256:258]
                        nc.tensor.matmul(y_ps, qs[:], Wc2a,
                                         start=True, stop=True)
                        nc.scalar.activation(
                            out_sb[:, b * 2:(b + 1) * 2], y_ps, Act.Copy)

                if layer == 0:
                    nfull = (NPC // 128) * 128
                    nc.sync.dma_start(
                        z1b[0:nfull, 0:64].rearrange("(b p) f -> p b f", p=128),
                        zstage[:, 0:(NPC // 128) * D])
                    nc.sync.dma_start(
                        z1b[nfull:NPC, 0:64],
                        zstage[0:NPC - nfull, (NPC // 128) * D:NBLK * D])
                    nc.gpsimd.collective_compute(
                        "AllGather", mybir.AluOpType.bypass,
                        replica_groups=[list(range(C))],
                        ins=[z1b[:].opt()], outs=[z1f[:].opt()])

            nc.sync.dma_start(out_d[:], out_sb[:])
            if DEBUG:
                nc.sync.dma_start(dbga_d[:], dbga_t[:])
    nc.compile()
    return nc


def _fold_bn(g, beta, rm, rv):
    a = g / np.sqrt(rv + BN_EPS)
    return a.astype(np.float32), (beta - rm * a).astype(np.float32)


def kernel(**inputs):
    from concourse.bass_utils import run_bass_kernel_spmd

    x = np.asarray(inputs["x"], np.float32)
    ei = np.asarray(inputs["edge_index"])
    meta = _preprocess(ei)

    key = (meta["nch"], tuple(meta["chunk_meta"]))
    if key not in _compiled:
        _compiled.clear()
        _compiled[key] = _build(meta)
    nc = _compiled[key]

    bf = ml_dtypes.bfloat16
    xpad = np.zeros((N, 128), bf)
    xpad[:, 0:64] = x.astype(bf)

    iota = np.tile(np.arange(128, dtype=np.float32)[None, :].astype(bf),
                   (128, G_ONEHOT))
    ident = np.eye(128, dtype=np.float32).astype(bf)

    W1 = np.asarray(inputs["W1"], np.float32)
    W2 = np.asarray(inputs["W2"], np.float32)
    Wc1 = np.asarray(inputs["Wc1"], np.float32)
    Wc2 = np.asarray(inputs["Wc2"], np.float32)
    wb = np.zeros((65, 168), np.float32)
    wb[0:64, 0:64] = W1            # W1_aug: col 64 stays 0
    wb[0:64, 65:129] = W2
    wb[0:64, 130:162] = Wc1        # Wc1_aug [65, 33]
    wb[64, 130:162] = np.asarray(inputs["bc1"], np.float32)
    wb[64, 162] = 1.0              # ones-passthrough column
    wb[0:32, 163:165] = Wc2        # Wc2_aug [33, 2]
    wb[32, 163:165] = np.asarray(inputs["bc2"], np.float32)
    wb = wb.astype(bf)

    a1, c1 = _fold_bn(*[np.asarray(inputs[k], np.float32)
                        for k in ("g1", "beta1", "rm1", "rv1")])
    a2, c2 = _fold_bn(*[np.asarray(inputs[k], np.float32)
                        for k in ("g2", "beta2", "rm2", "rv2")])
    vecs = np.zeros((65, 8), np.float32)
    vecs[0:64, 0] = np.asarray(inputs["b1"], np.float32)
    vecs[0:64, 1], vecs[0:64, 2] = a1, c1
    vecs[0:64, 3] = np.asarray(inputs["b2"], np.float32)
    vecs[0:64, 4], vecs[0:64, 5] = a2, c2
    # constant-one row rides through bias/scale of both Prelus
    vecs[64, 0] = 1.0   # z row64 = 0 -> +1 -> Prelu = 1
    vecs[64, 1] = 1.0   # 1*1 + 0 -> Prelu = 1
    vecs[64, 2] = 0.0
    vecs[64, 3] = 1.0
    vecs[64, 4] = 1.0
    vecs[64, 5] = 0.0

    in_maps = []
    for c in range(C):
        in_maps.append({
            "xpad": xpad, "idx": meta["idx_t"][c], "dl": meta["dl_t"][c],
            "iota": iota, "ident": ident, "wb": wb, "vecs": vecs,
            "s": meta["s_t"][c],
        })
    res = run_bass_kernel_spmd(nc, in_maps, core_ids=list(range(C)))
    global _last_res, _last_meta
    _last_res, _last_meta = res, meta
    outs = []
    for c in range(C):
        o = res.results[c]["out"]                      # [128, 2*NBLK]
        o = o.reshape(128, NBLK, 2).transpose(1, 0, 2).reshape(NPAD, 2)
        outs.append(o[0:NPC])
    return np.concatenate(outs, axis=0).astype(np.float32)
